# revision 59
# baseline (speedup 1.0000x reference)
"""Trainium2 Bass kernel for nn_BlurLayer: batched FFT2D low-pass filter bank.

Math: for each 256x256 image X, each cutoff u, the reference computes
Re(IFFT2(ifftshift(mask_u * fftshift(FFT2(X))))) with mask_u a centered
(2u+1)^2 block of ones.  That ideal low-pass filter is separable and equals
D_u @ X @ D_u with the real symmetric projection D_u = G_u G_u^T, where the
columns of G are the orthonormal real Fourier vectors ordered by |frequency|
(DC, cos1, sin1, ..., cos127, sin127, 0-pad), and G_u = G[:, :2u+1].

The kernel evaluates, per image, the shared spectral analysis

    out1 = matmul(lhsT=X,    rhs=G)    # X^T G            (stage A)
    S    = matmul(lhsT=out1, rhs=G)    # G^T X G          (stage B)

and per cutoff u the rank-(2u+1) synthesis (r = 2u+1)

    out3 = matmul(lhsT=S[:r,:r], rhs=G^T[:r])   # S_u^T G_u^T   (stage C)
    out  = matmul(lhsT=out3,     rhs=G^T[:r])   # G_u S_u G_u^T (stage D)

All four are plain TensorE matmuls chained through the stationary operand
(out = lhsT.T @ rhs), so no transposes are needed and the result lands as
[w, h], exactly the output layout.  Data parallel over 8 NeuronCores, 8
images per core.  The device writes the 8 filtered channels planar
([n, w, c, h]); channel interleave and the channel-0 passthrough are
assembled on the host.
"""

import os
import sys

import numpy as np

for _p in ("/opt/trn_rl_repo", "/root/.axon_site/_ro/trn_rl_repo"):
    if os.path.isdir(_p) and _p not in sys.path:
        sys.path.insert(0, _p)

import concourse.bass as bass
import concourse.mybir as mybir
from concourse.tile import TileContext
from concourse.bass_utils import run_bass_kernel_spmd

# ---------------------------------------------------------------- problem spec
SIZE = 256
N_IMG = 64
N_CORES = 8
PER_CORE = N_IMG // N_CORES  # 8 images per core
US = [int(v) for v in np.linspace(5.0, SIZE // 2 - 1.0, 8)]  # [5,...,127]
N_U = len(US)
N_PAIR = N_U // 2

# matmul operand dtype: "f32r" (full-rate fp32 mode), "f32" (exact, 4x slower)
MM_MODE = os.environ.get("BLUR_MM_MODE", "f32r")
ALGO = os.environ.get("BLUR_ALGO", "fold")  # "fold", "direct", "batch", or "g"

_F32 = mybir.dt.float32
_F32R = mybir.dt.float32r
_F16 = mybir.dt.float16
_MM_DT = _F32R if MM_MODE == "f32r" else _F32

# ---- folded-parity algorithm ("fold") constants -----------------------------
# Channels 1..7 (u in US[:7]) run on device; channel 8 (u=127) is I minus the
# Nyquist rank-1 projector, reconstructed exactly on the host; channel 0 is the
# passthrough.  Each 256x256 image folds under the reflection p <-> 255-p into
# 4 parity quadrants of exactly 128x128 (the half-sample cos/sin eigenbasis of
# any symmetric circulant filter splits 128/128 with no fixed points), and
# D_u X D_u decomposes into independent per-quadrant products
#     out_ab = De/o_u @ X_ab @ De/o_u            (a, b = row/col parity)
# with folded filters  Dpar_u = 0.5 * (k(p-q) +/- k(p+q+1)),  k = Dirichlet.
# The 0.5 per side absorbs the 1/4 of the two-sided unfold.
N_UD = 7  # device channels
US_DEV = US[:N_UD]
# channels shipped as fp8e4m3 instead of fp16 (disabled: the ~1us DMA saving
# is not worth shrinking the 33x error margin to 3x)
N_F8 = 0


def _build_g_matrices():
    """G [space, col] with cols (DC, cos1, sin1, ..., cos127, sin127, 0).
    Returns (g, gt): g[k, p, c] = G[128k+p, c]; gt[k, p, c] = G^T[128k+p, c]."""
    a = np.arange(SIZE)
    cols = [np.full(SIZE, 1.0 / np.sqrt(SIZE))]
    for f in range(1, 128):
        cols.append(np.sqrt(2.0 / SIZE) * np.cos(2 * np.pi * f * a / SIZE))
        cols.append(np.sqrt(2.0 / SIZE) * np.sin(2 * np.pi * f * a / SIZE))
    cols.append(np.zeros(SIZE))
    G = np.stack(cols, axis=1).astype(np.float32)
    g = np.stack([G[:128], G[128:]])
    GT = np.ascontiguousarray(G.T)
    gt = np.stack([GT[:128], GT[128:]])
    return g, gt


def _build_d_matrices() -> np.ndarray:
    """d[p, u*256+j] = D_u[p, j] (top half only; the bottom half is the
    128-column rotation, derived on device), float32, [128, 8*256]."""
    a = np.arange(SIZE)
    diff = a[:, None] - a[None, :]
    d = np.empty((128, N_U * SIZE), np.float32)
    for ui, u in enumerate(US):
        f = np.arange(1, u + 1)
        acc = np.ones((SIZE, SIZE), np.float64)
        ang = 2.0 * np.pi * diff[..., None] * f / SIZE
        acc += 2.0 * np.cos(ang).sum(axis=-1)
        Du = (acc / SIZE).astype(np.float32)
        d[:, ui * SIZE:(ui + 1) * SIZE] = Du[:128]
    return d


def _dirichlet(t: np.ndarray, u: int) -> np.ndarray:
    """Closed-form symmetric ideal low-pass kernel k_u(t), period 256."""
    t = np.asarray(t, np.float64)
    s = np.sin(np.pi * t / SIZE)
    with np.errstate(divide="ignore", invalid="ignore"):
        k = np.sin(np.pi * (2 * u + 1) * t / SIZE) / (SIZE * s)
    return np.where(np.abs(s) < 1e-12, (2 * u + 1) / SIZE, k)


def _build_fold_d() -> np.ndarray:
    """Folded filter banks, 0.5-scaled: d[par, p, u*128 + q], fp16 [2,128,896]."""
    p = np.arange(128)
    diff = p[:, None] - p[None, :]
    ssum = p[:, None] + p[None, :] + 1
    d = np.empty((2, 128, N_UD * 128), np.float16)
    for ui, u in enumerate(US_DEV):
        kd = _dirichlet(diff, u)
        ks = _dirichlet(ssum, u)
        d[0, :, ui * 128:(ui + 1) * 128] = (0.5 * (kd + ks)).astype(np.float16)
        d[1, :, ui * 128:(ui + 1) * 128] = (0.5 * (kd - ks)).astype(np.float16)
    return d


def _fold_x(xs: np.ndarray) -> np.ndarray:
    """Fold a core's images [8,256,256] f32 into quadrants.
    Returns [2, 128, 8*2*128] fp16 laid out [a, wa, (img, b, hb)]."""
    xr = [xs[:, :128, :] + xs[:, ::-1, :][:, :128, :],
          xs[:, :128, :] - xs[:, ::-1, :][:, :128, :]]
    out = np.empty((2, 128, PER_CORE, 2, 128), np.float16)
    for a in range(2):
        for b in range(2):
            sb = 1.0 if b == 0 else -1.0
            q = xr[a][:, :, :128] + sb * xr[a][:, :, ::-1][:, :, :128]
            out[a, :, :, b, :] = q.transpose(1, 0, 2)
    return out.reshape(2, 128, PER_CORE * 2 * 128)


def _build_program_fold() -> bass.Bass:
    """Folded-parity fp16 pipeline.

    stage 1 (per img, a, b):  M1 = X_ab^T @ D_ua     [hb, space-a]
    stage 2 (per u, b):       outT = D_ub @ M1       [space-b, (img, a, space-a)]
    lhsT is always a [128,128] fp16 stationary (FWL-eligible); stage-2 streams
    512-col chunks of many images per weight load.  PSUM is drained by vector
    and scalar alternately (the binding resource); output leaves as fp16."""
    nc = bass.Bass()
    _F8 = mybir.dt.float8e4
    x_dram = nc.declare_dram_parameter("x", [2, 128, PER_CORE * 2 * 128], _F16, isOutput=False)
    d_dram = nc.declare_dram_parameter("d", [2, 128, N_UD * 128], _F16, isOutput=False)
    # flat pair-stream layout: chunks land in drain-completion order, two per
    # 4KB/partition row for DMA packet efficiency; host decodes the fixed order
    o_dram = nc.declare_dram_parameter(
        "out", [2 * N_UD, 128, 2048], _F16, isOutput=True)

    with TileContext(nc) as tc:
        with (
            tc.tile_pool(name="xin", bufs=1) as xin_pool,
            tc.tile_pool(name="dmat", bufs=1) as d_pool,
            tc.tile_pool(name="m1", bufs=1) as m1_pool,
            tc.tile_pool(name="ot", bufs=10) as ot_pool,
            tc.tile_pool(name="scr", bufs=1) as scr_pool,
            tc.tile_pool(name="ps", bufs=4, space="PSUM") as ps_pool,
        ):
            # scratch for PE warm-up (memset first so warm-up can start early)
            scr = scr_pool.tile([128, 512], _F16, tag="scr", name="scr")
            nc.vector.memset(scr[:], 0.0)

            # input DMAs: one per engine ring so each is first in its ring and
            # descriptor generation runs in parallel; tensor/vector rings come
            # out of the engine prologue earliest.
            xa = [None, None]
            for a in range(2):
                xa[a] = xin_pool.tile([128, PER_CORE * 2 * 128], _F16,
                                      tag=f"x{a}", name=f"x_{a}")
            dt = [None, None]
            for par in range(2):
                dt[par] = d_pool.tile([128, N_UD * 128], _F16, tag=f"d{par}",
                                      name=f"d_{par}")
            # d on the otherwise-idle scalar ring; x quartered so the first
            # stage-1 blocks can start as soon as imgs 0-1 land
            nc.scalar.dma_start(out=dt[0][:], in_=d_dram[0])
            nc.scalar.dma_start(out=dt[1][:], in_=d_dram[1])
            nc.sync.dma_start(out=xa[0][:, 0:512], in_=x_dram[0][:, 0:512])
            nc.gpsimd.dma_start(out=xa[1][:, 0:512], in_=x_dram[1][:, 0:512])
            nc.sync.dma_start(out=xa[0][:, 512:1024], in_=x_dram[0][:, 512:1024])
            nc.gpsimd.dma_start(out=xa[1][:, 512:1024], in_=x_dram[1][:, 512:1024])
            nc.sync.dma_start(out=xa[0][:, 1024:2048], in_=x_dram[0][:, 1024:2048])
            nc.gpsimd.dma_start(out=xa[1][:, 1024:2048], in_=x_dram[1][:, 1024:2048])
            m1 = [None, None]
            for b in range(2):
                m1[b] = m1_pool.tile([128, N_UD, PER_CORE * 2 * 128], _F16,
                                     tag=f"m1{b}", name=f"m1_{b}")

            # PE warm-up: HAM un-throttles after ~3.4us of sustained matmul
            # activity; burn the input-DMA wait on dummy matmuls over the
            # memset scratch tile so stage 1 starts at 2.4 GHz.
            s0 = ps_pool.tile([128, 1024], _F32, tag="ps", name="s_warm")
            for wi in range(11):
                nc.tensor.matmul(s0[:, 0:512], scr[:, 0:128], scr[:],
                                 start=True, stop=True, skip_group_check=True)

            drains = [0]

            def drain(dst, src):
                if drains[0] % 2 == 0:
                    nc.scalar.copy(dst, src)
                else:
                    nc.vector.tensor_copy(dst, src)
                drains[0] += 1

            def s1_block(img, a, b):
                s1 = ps_pool.tile([128, 1024], _F32, tag="ps",
                                  name=f"s1_{img}_{a}_{b}")
                lhsT = xa[a][:, (img * 2 + b) * 128:(img * 2 + b + 1) * 128]
                nc.tensor.matmul(s1[:, 0:512], lhsT, dt[a][:, 0:512],
                                 start=True, stop=True, skip_group_check=True)
                nc.tensor.matmul(s1[:, 512:N_UD * 128], lhsT,
                                 dt[a][:, 512:N_UD * 128],
                                 start=True, stop=True, skip_group_check=True)
                dst = m1[b].rearrange("p u (i c) -> p u i c", c=128)[
                    :, :, img * 2 + a, :]
                src = s1[:, 0:N_UD * 128].rearrange("p (u c) -> p u c", c=128)
                drain(dst, src)

            rings = [0]

            pair = [None]

            def s2_chunk(u, b, half):
                lhsT = dt[b][:, u * 128:(u + 1) * 128]
                s2 = ps_pool.tile([128, 1024], _F32, tag="ps",
                                  name=f"s2_{half}_{u}_{b}")
                for k2 in range(2):
                    rhs = m1[b][:, u, 1024 * half + 512 * k2:
                                1024 * half + 512 * (k2 + 1)]
                    nc.tensor.matmul(s2[:, 512 * k2:512 * (k2 + 1)], lhsT, rhs,
                                     start=True, stop=True, skip_group_check=True)
                ci = rings[0]
                rings[0] += 1
                if ci % 2 == 0:
                    pair[0] = ot_pool.tile([128, 2048], _F16, tag="ot",
                                           name=f"ot_p{ci // 2}")
                ot = pair[0]
                drain(ot[:, 1024 * (ci % 2):1024 * (ci % 2 + 1)], s2[:])
                if ci % 2 == 1:
                    p = ci // 2
                    if p >= 12:
                        # final pairs: halve transfer latency by splitting each
                        # across two rings in parallel
                        nc.sync.dma_start(out=o_dram[p][:, 0:1024],
                                          in_=ot[:, 0:1024])
                        nc.gpsimd.dma_start(out=o_dram[p][:, 1024:2048],
                                            in_=ot[:, 1024:2048])
                    else:
                        dma_eng = (nc.sync, nc.gpsimd, nc.scalar, nc.sync,
                                   nc.gpsimd)[p % 5]
                        dma_eng.dma_start(out=o_dram[p], in_=ot[:])

            def interleave(s1_args, s2_args):
                s1_it, s2_it = iter(s1_args), iter(s2_args)
                while True:
                    done = 0
                    for it, fn in ((s1_it, s1_block), (s2_it, s2_chunk)):
                        try:
                            fn(*next(it))
                        except StopIteration:
                            done += 1
                    if done == 2:
                        break

            # b-split schedule: stage-2 output production starts after only 8
            # stage-1 blocks and stays roughly uniform, so the output DMA
            # rings (the end-to-end critical path) run from ~18us onward.
            h0, h1 = (0, 1, 2, 3), (4, 5, 6, 7)
            interleave([(i, a, 0) for i in h0 for a in range(2)], [])
            interleave([(i, a, 1) for i in h0 for a in range(2)],
                       [(u, 0, 0) for u in range(N_UD)])
            interleave([(i, a, 0) for i in h1 for a in range(2)],
                       [(u, 1, 0) for u in range(N_UD)])
            interleave([(i, a, 1) for i in h1 for a in range(2)],
                       [(u, 0, 1) for u in range(N_UD)])
            interleave([], [(u, 1, 1) for u in range(N_UD)])

    _split_sync_waits(nc, max_waits=1)
    return nc


def _split_sync_waits(nc, max_waits=1):
    """Walrus in this container only accepts 1 sync-wait per instruction;
    hoist excess waits onto same-engine NOPs inserted just before."""
    for f in nc.m.functions:
        for bb in f.blocks:
            insts = bb.instructions
            i = 0
            while i < len(insts):
                inst = insts[i]
                si = inst.sync_info
                if si is not None and si.on_wait and len(si.on_wait) > max_waits:
                    waits = list(si.on_wait)
                    keep = waits[-max_waits:]
                    excess = waits[:-max_waits]
                    si.on_wait = keep
                    eng = nc.engines[inst.engine]
                    new_nops = []
                    for j in range(0, len(excess), max_waits):
                        chunk = excess[j:j + max_waits]
                        nop = eng.nop(nofuse=True, hint=f"wsplit_{inst.name}_{j}")
                        nop_inst = nop.ins if hasattr(nop, "ins") else nop
                        for f2 in nc.m.functions:
                            for bb2 in f2.blocks:
                                if nop_inst in bb2.instructions and not (
                                    bb2 is bb and bb2.instructions.index(nop_inst) < i
                                ):
                                    bb2.instructions.remove(nop_inst)
                        if nop_inst.sync_info is None:
                            nop_inst.sync_info = mybir.SyncInfo(
                                on_wait=chunk, on_update=[]
                            )
                        else:
                            nop_inst.sync_info.on_wait = chunk
                        new_nops.append(nop_inst)
                    for k, nop_inst in enumerate(new_nops):
                        insts.insert(i + k, nop_inst)
                    i += len(new_nops)
                i += 1


def _strip_redundant_mm_incs(nc):
    """Drop then_inc updates on matmuls where they are provably unobserved.
    Serialized sem increments cost ~26ns each on the PE.  An increment is
    kept iff it belongs to a stop matmul OR some wait references its exact
    cumulative value (this includes the same-engine PSUM-WAR guards that
    deadlocked the naive round-up version).  All awaited values then map
    exactly onto retained increments, so no wait can move past its original
    producer."""
    import concourse.mybir as mb

    pe_sem_id = None
    inc_events = []
    for f in nc.m.functions:
        for bb in f.blocks:
            for inst in bb.instructions:
                si = inst.sync_info
                if not (isinstance(inst, mb.InstMatmult) and si and si.on_update):
                    continue
                for upd in si.on_update:
                    uid = getattr(upd, "id", None)
                    if pe_sem_id is None:
                        pe_sem_id = uid
                    if uid == pe_sem_id:
                        inc_events.append((inst, upd))
    if pe_sem_id is None:
        return 0
    # all waits on this sem; abort on anything but simple sem-ge-imm
    awaited = set()
    for f in nc.m.functions:
        for bb in f.blocks:
            for inst in bb.instructions:
                si = inst.sync_info
                if si and si.on_wait:
                    for w in si.on_wait:
                        if getattr(w, "id", None) == pe_sem_id:
                            if w.wait_mode != "sem-ge-imm" or w.wait_reg is not None:
                                return 0
                            awaited.add(w.wait_value)
    keep_flags = []
    for v, (inst, _upd) in enumerate(inc_events, start=1):
        keep_flags.append(bool(inst.stop_tensor_calc) or v in awaited)
    new_of_old = {}
    kept = 0
    for v, k in enumerate(keep_flags, start=1):
        if k:
            kept += 1
        new_of_old[v] = kept
    if any(v not in new_of_old or not keep_flags[v - 1] for v in awaited):
        return 0  # paranoia: every awaited value must be a retained inc
    for f in nc.m.functions:
        for bb in f.blocks:
            for inst in bb.instructions:
                si = inst.sync_info
                if si and si.on_wait:
                    for w in si.on_wait:
                        if getattr(w, "id", None) == pe_sem_id:
                            w.wait_value = new_of_old[w.wait_value]
    n_dropped = 0
    for (inst, upd), k in zip(inc_events, keep_flags):
        if not k:
            inst.sync_info.on_update = [
                u for u in inst.sync_info.on_update if u is not upd
            ]
            n_dropped += 1
    return n_dropped


def _build_program_g() -> bass.Bass:
    nc = bass.Bass()
    x_dram = nc.declare_dram_parameter("x", [PER_CORE, SIZE, SIZE], _MM_DT, isOutput=False)
    g_dram = nc.declare_dram_parameter("g", [2, 128, SIZE], _MM_DT, isOutput=False)
    gt_dram = nc.declare_dram_parameter("gt", [2, 128, SIZE], _MM_DT, isOutput=False)
    # planar channel layout [n, w, c, h]; host reorders to [n, w, h, c]
    o_dram = nc.declare_dram_parameter("out", [PER_CORE, SIZE, N_U, SIZE], _F32, isOutput=True)

    with TileContext(nc) as tc:
        with (
            tc.tile_pool(name="xin", bufs=2 * PER_CORE) as xin_pool,
            tc.tile_pool(name="gmat", bufs=4) as g_pool,
            tc.tile_pool(name="oA", bufs=2) as oA_pool,
            tc.tile_pool(name="oS", bufs=2) as oS_pool,
            tc.tile_pool(name="oC", bufs=6) as oC_pool,
            tc.tile_pool(name="obig", bufs=4) as obig_pool,
            tc.tile_pool(name="psAB", bufs=3, space="PSUM") as psAB_pool,
            tc.tile_pool(name="psC", bufs=2, space="PSUM") as psC_pool,
            tc.tile_pool(name="psD", bufs=3, space="PSUM") as psD_pool,
        ):
            # G/GT tiles first (small, gate the first matmuls)
            g_t, gt_t = [None, None], [None, None]
            for k in range(2):
                g_t[k] = g_pool.tile([128, SIZE], _MM_DT, tag="g", name=f"g_{k}")
                nc.sync.dma_start(out=g_t[k][:], in_=g_dram[k])
            for k in range(2):
                gt_t[k] = g_pool.tile([128, SIZE], _MM_DT, tag="gt", name=f"gt_{k}")
                nc.sync.dma_start(out=gt_t[k][:], in_=gt_dram[k])

            # X tiles on the gpsimd (SWDGE) queue so they don't serialize
            # behind output DMAs on the sync queue
            x_t = [[None] * PER_CORE for _ in range(2)]
            for n in range(PER_CORE):
                for k in range(2):
                    t = xin_pool.tile([128, SIZE], _MM_DT, tag=f"x{k}", name=f"x_{k}_{n}")
                    nc.gpsimd.dma_start(out=t[:], in_=x_dram[n, k * 128:(k + 1) * 128, :])
                    x_t[k][n] = t

            for n in range(PER_CORE):
                # ---- stage A: out1 = X^T G, h-blocks in free halves
                sA = psAB_pool.tile([128, 512], _F32, tag="sAB", name=f"sA_{n}")
                for m in range(2):
                    for k in range(2):
                        nc.tensor.matmul(
                            sA[:, m * 256:(m + 1) * 256],
                            x_t[k][n][:, m * 128:(m + 1) * 128],
                            g_t[k][:],
                            start=(k == 0),
                            stop=(k == 1),
                            skip_group_check=True,
                        )
                oA = oA_pool.tile([128, 512], _MM_DT, tag="oA", name=f"oA_{n}")
                nc.vector.tensor_copy(oA[:], sA[:])

                # ---- stage B: S = G^T X G, f1-blocks in free halves
                sB = psAB_pool.tile([128, 512], _F32, tag="sAB", name=f"sB_{n}")
                for mB in range(2):
                    for kB in range(2):
                        nc.tensor.matmul(
                            sB[:, mB * 256:(mB + 1) * 256],
                            oA[:, kB * 256 + mB * 128: kB * 256 + (mB + 1) * 128],
                            g_t[kB][:],
                            start=(kB == 0),
                            stop=(kB == 1),
                            skip_group_check=True,
                        )
                oS = oS_pool.tile([128, 512], _MM_DT, tag="oS", name=f"oS_{n}")
                nc.scalar.copy(oS[:], sB[:])

                # ---- stages C+D per pair of cutoffs
                out_big = [
                    obig_pool.tile([128, N_U, SIZE], _F32, tag="ob", name=f"ob_{n}_{m2b}")
                    for m2b in range(2)
                ]
                for pr in range(N_PAIR):
                    oC = [None, None]
                    for ha in range(2):
                        u = US[2 * pr + ha]
                        r = 2 * u + 1
                        nblk = 1 if r <= 128 else 2
                        sC = psC_pool.tile([128, 512], _F32, tag="sC", name=f"sC_{n}_{pr}_{ha}")
                        for m3 in range(nblk):
                            m3w = min(128, r - m3 * 128)
                            for c1 in range(nblk):
                                c1w = min(128, r - c1 * 128)
                                nc.tensor.matmul(
                                    sC[0:m3w, m3 * 256:m3 * 256 + 256],
                                    oS[0:c1w, c1 * 256 + m3 * 128: c1 * 256 + m3 * 128 + m3w],
                                    gt_t[c1][0:c1w, :],
                                    start=(c1 == 0),
                                    stop=(c1 == nblk - 1),
                                    skip_group_check=True,
                                )
                        oCt = oC_pool.tile([128, 512], _MM_DT, tag="oC", name=f"oC_{n}_{pr}_{ha}")
                        if ha == 0:
                            nc.vector.tensor_copy(oCt[:, 0:256 * nblk], sC[:, 0:256 * nblk])
                        else:
                            nc.scalar.copy(oCt[:, 0:256 * nblk], sC[:, 0:256 * nblk])
                        oC[ha] = oCt

                    for m2 in range(2):
                        sD = psD_pool.tile([128, 2, SIZE], _F32, tag="sD", name=f"sD_{n}_{pr}_{m2}")
                        for ha in range(2):
                            u = US[2 * pr + ha]
                            r = 2 * u + 1
                            nkD = 1 if r <= 128 else 2
                            for kD in range(nkD):
                                kw = min(128, r - kD * 128)
                                nc.tensor.matmul(
                                    sD[:, ha, :],
                                    oC[ha][0:kw, kD * 256 + m2 * 128: kD * 256 + m2 * 128 + 128],
                                    gt_t[kD][0:kw, :],
                                    start=(kD == 0),
                                    stop=(kD == nkD - 1),
                                    skip_group_check=True,
                                )
                        dst = out_big[m2][:, 2 * pr:2 * pr + 2, :]
                        if m2 == 0:
                            nc.vector.tensor_copy(dst, sD[:])
                        else:
                            nc.scalar.copy(dst, sD[:])

                for m2 in range(2):
                    nc.sync.dma_start(
                        out=o_dram[n, m2 * 128:(m2 + 1) * 128, :, :],
                        in_=out_big[m2][:],
                    )

    _split_sync_waits(nc, max_waits=1)
    return nc


def _build_program_batch() -> bass.Bass:
    """Direct algorithm with stage-2 flipped: D_u stationary, o1 moving with
    TWO images batched per rhs (N=512 everywhere, 32 MMs/image instead of 48).
    Stage-2 output comes out transposed (out_u^T), so the device writes
    [n, h, c, w] planar and the host transposes to [n, w, h, c]."""
    nc = bass.Bass()
    x_dram = nc.declare_dram_parameter("x", [PER_CORE, SIZE, SIZE], _MM_DT, isOutput=False)
    d_dram = nc.declare_dram_parameter("d", [128, N_U * SIZE], _MM_DT, isOutput=False)
    o_dram = nc.declare_dram_parameter("out", [PER_CORE, SIZE, N_U, SIZE], _F32, isOutput=True)

    with TileContext(nc) as tc:
        with (
            tc.tile_pool(name="xin", bufs=2 * PER_CORE) as xin_pool,
            tc.tile_pool(name="dmat", bufs=2 * N_PAIR) as d_pool,
            tc.tile_pool(name="o1", bufs=6) as o1_pool,
            tc.tile_pool(name="obig", bufs=6) as obig_pool,
            tc.tile_pool(name="ps1", bufs=2, space="PSUM") as ps1_pool,
            tc.tile_pool(name="ps2", bufs=4, space="PSUM") as ps2_pool,
        ):
            d_t = [[None] * N_PAIR for _ in range(2)]
            x_t = [[None] * PER_CORE for _ in range(2)]

            def load_d(k, pr):
                if k == 0:
                    t = d_pool.tile([128, 512], _MM_DT, tag="d0", name=f"d_0_{pr}")
                    nc.sync.dma_start(out=t[:], in_=d_dram[:, pr * 512:(pr + 1) * 512])
                    d_t[0][pr] = t
                else:
                    t = d_pool.tile([128, 512], _MM_DT, tag="d1", name=f"d_1_{pr}")
                    d0 = d_t[0][pr]
                    for ha in range(2):
                        b = ha * 256
                        nc.vector.tensor_copy(
                            t[:, b:b + 128], d0[:, b + 128:b + 256].bitcast(_F32))
                        nc.vector.tensor_copy(
                            t[:, b + 128:b + 256], d0[:, b:b + 128].bitcast(_F32))
                    d_t[1][pr] = t

            def load_x(k, n, eng):
                t = xin_pool.tile([128, SIZE], _MM_DT, tag=f"x{k}", name=f"x_{k}_{n}")
                eng.dma_start(out=t[:], in_=x_dram[n, k * 128:(k + 1) * 128, :])
                x_t[k][n] = t

            load_d(0, 0)
            load_x(0, 0, nc.gpsimd)
            load_d(1, 0)
            load_x(1, 0, nc.gpsimd)
            for pr in range(1, N_PAIR):
                load_d(0, pr)
                load_d(1, pr)
            for n in range(1, PER_CORE):
                load_x(0, n, nc.gpsimd)
                load_x(1, n, nc.gpsimd)

            for ip in range(PER_CORE // 2):
                nA, nB = 2 * ip, 2 * ip + 1
                # ---- stage 1: o1g[p, kp*1024 + img*512 + paircol]
                #      = (X_img^T D_pair)[kp*128+p, paircol]
                o1g = [None] * N_PAIR
                for pr in range(N_PAIR):
                    o1gt = o1_pool.tile([128, 2048], _MM_DT, tag="o1", name=f"o1_{ip}_{pr}")
                    for kp in range(2):
                        s1 = ps1_pool.tile([128, 1024], _F32, tag="s1", name=f"s1_{ip}_{pr}_{kp}")
                        for img, n in enumerate((nA, nB)):
                            for k in range(2):
                                nc.tensor.matmul(
                                    s1[:, img * 512:(img + 1) * 512],
                                    x_t[k][n][:, kp * 128:(kp + 1) * 128],
                                    d_t[k][pr][:],
                                    start=(k == 0),
                                    stop=(k == 1),
                                    skip_group_check=True,
                                )
                        dst = o1gt[:, kp * 1024:(kp + 1) * 1024]
                        if (pr + kp) % 2 == 0:
                            nc.vector.tensor_copy(dst, s1[:])
                        else:
                            nc.scalar.copy(dst, s1[:])
                    o1g[pr] = o1gt

                # ---- stage 2: D stationary, both images moving (N=512)
                # psum = out_u^T blocks: [mj(part) = h-axis, (img, w)]
                ob = [
                    [
                        obig_pool.tile([128, 2, N_U // 2, SIZE], _F32, tag="ob",
                                       name=f"ob_{ip}_{m}_{hb}")
                        for hb in range(2)
                    ]
                    for m in range(2)
                ]
                for pr in range(N_PAIR):
                    hb = pr // 2
                    for ha in range(2):
                        ci = (2 * pr + ha) % 4
                        for m in range(2):
                            s2 = ps2_pool.tile([128, 2, 256], _F32, tag="s2",
                                               name=f"s2_{ip}_{pr}_{ha}_{m}")
                            o1v = o1g[pr].rearrange("p (a b c) -> p a b c", a=2, b=2, c=512)
                            for kp in range(2):
                                lhsT = d_t[kp][pr][:, ha * 256 + m * 128:
                                                   ha * 256 + (m + 1) * 128]
                                rhs = o1v[:, kp, :, ha * 256:(ha + 1) * 256]
                                nc.tensor.matmul(
                                    s2[:],
                                    lhsT,
                                    rhs,
                                    start=(kp == 0),
                                    stop=(kp == 1),
                                    skip_group_check=True,
                                )
                            dst = ob[m][hb][:, :, ci, :]
                            if m == 0:
                                nc.vector.tensor_copy(dst, s2[:])
                            else:
                                nc.scalar.copy(dst, s2[:])
                    if pr % 2 == 1:
                        for m in range(2):
                            for img, n in enumerate((nA, nB)):
                                nc.sync.dma_start(
                                    out=o_dram[n, m * 128:(m + 1) * 128,
                                               hb * 4:(hb + 1) * 4, :],
                                    in_=ob[m][hb][:, img, :, :],
                                )

    _split_sync_waits(nc, max_waits=1)
    return nc


def _build_program_direct() -> bass.Bass:
    nc = bass.Bass()
    x_dram = nc.declare_dram_parameter("x", [PER_CORE, SIZE, SIZE], _MM_DT, isOutput=False)
    d_dram = nc.declare_dram_parameter("d", [128, N_U * SIZE], _MM_DT, isOutput=False)
    o_dram = nc.declare_dram_parameter("out", [PER_CORE, SIZE, N_U, SIZE], _F32, isOutput=True)

    with TileContext(nc) as tc:
        with (
            tc.tile_pool(name="xin", bufs=2 * PER_CORE) as xin_pool,
            tc.tile_pool(name="dmat", bufs=2 * N_PAIR) as d_pool,
            tc.tile_pool(name="o1", bufs=2 * N_PAIR) as o1_pool,
            tc.tile_pool(name="obig", bufs=8) as obig_pool,
            tc.tile_pool(name="ps1", bufs=5, space="PSUM") as ps1_pool,
            tc.tile_pool(name="ps2", bufs=3, space="PSUM") as ps2_pool,
        ):
            # interleave input DMAs so the earliest-needed tiles land first:
            # pair-0 D blocks and image-0 X blocks ahead of everything else
            d_t = [[None] * N_PAIR for _ in range(2)]
            x_t = [[None] * PER_CORE for _ in range(2)]

            def load_d(k, pr):
                if k == 0:
                    t = d_pool.tile([128, 512], _MM_DT, tag="d0", name=f"d_0_{pr}")
                    nc.sync.dma_start(out=t[:], in_=d_dram[:, pr * 512:(pr + 1) * 512])
                    d_t[0][pr] = t
                else:
                    t = d_pool.tile([128, 512], _MM_DT, tag="d1", name=f"d_1_{pr}")
                    d0 = d_t[0][pr]
                    for ha in range(2):
                        b = ha * 256
                        nc.vector.tensor_copy(
                            t[:, b:b + 128],
                            d0[:, b + 128:b + 256].bitcast(_F32),
                        )
                        nc.vector.tensor_copy(
                            t[:, b + 128:b + 256],
                            d0[:, b:b + 128].bitcast(_F32),
                        )
                    d_t[1][pr] = t

            def load_x(n):
                # one DMA per image: tile [p, k, h] <- x[n, k*128+p, h]
                t = xin_pool.tile([128, 2, SIZE], _MM_DT, tag="x", name=f"x_{n}")
                nc.gpsimd.dma_start(
                    out=t[:], in_=x_dram[n].rearrange("(k p) h -> p k h", k=2))
                x_t[0][n] = t

            load_d(0, 0)
            load_x(0)
            load_d(1, 0)
            for pr in range(1, N_PAIR):
                load_d(0, pr)
                load_d(1, pr)
            for n in range(1, PER_CORE):
                load_x(n)

            for n in range(PER_CORE):
                o1 = [None] * N_PAIR
                for pr in range(N_PAIR):
                    s1h = []
                    for m in range(2):
                        s1 = ps1_pool.tile([128, 512], _F32, tag="s1", name=f"s1_{n}_{pr}_{m}")
                        for k in range(2):
                            nc.tensor.matmul(
                                s1[:],
                                x_t[0][n][:, k, m * 128:(m + 1) * 128],
                                d_t[k][pr][:],
                                start=(k == 0),
                                stop=(k == 1),
                                skip_group_check=True,
                            )
                        s1h.append(s1)
                    o1t = o1_pool.tile([128, 1024], _MM_DT, tag="o1", name=f"o1_{n}_{pr}")
                    for m in range(2):
                        if (pr + m) % 2 == 0:
                            nc.vector.tensor_copy(o1t[:, m * 512:(m + 1) * 512], s1h[m][:])
                        else:
                            nc.scalar.copy(o1t[:, m * 512:(m + 1) * 512], s1h[m][:])
                    o1[pr] = o1t

                # two half-tiles per w-block: channels 0-3 from pairs 0-1,
                # channels 4-7 from pairs 2-3 -> DMA each half when ready
                last = n == PER_CORE - 1
                out_half = [
                    [
                        obig_pool.tile([128, N_U // 2, SIZE], _F32, tag="ob", name=f"ob_{n}_{m2b}_{hb}")
                        for hb in range(2)
                    ]
                    for m2b in range(2)
                ]
                for pr in range(N_PAIR):
                    hb = pr // 2
                    for m2 in range(2):
                        s2 = ps2_pool.tile([128, 2, SIZE], _F32, tag="s2", name=f"s2_{n}_{pr}_{m2}")
                        for ha in range(2):
                            for kp in range(2):
                                lhs = o1[pr][:, kp * 512 + ha * 256 + m2 * 128:
                                             kp * 512 + ha * 256 + (m2 + 1) * 128]
                                rhs = d_t[kp][pr][:, ha * 256:(ha + 1) * 256]
                                nc.tensor.matmul(
                                    s2[:, ha, :],
                                    lhs,
                                    rhs,
                                    start=(kp == 0),
                                    stop=(kp == 1),
                                    skip_group_check=True,
                                )
                        dst = out_half[m2][hb][:, (2 * pr) % 4:(2 * pr) % 4 + 2, :]
                        if m2 == 0:
                            nc.vector.tensor_copy(dst, s2[:])
                        else:
                            nc.scalar.copy(dst, s2[:])
                    if last:
                        for m2 in range(2):
                            nc.sync.dma_start(
                                out=o_dram[n, m2 * 128:(m2 + 1) * 128,
                                           2 * pr:2 * pr + 2, :],
                                in_=out_half[m2][hb][:, (2 * pr) % 4:(2 * pr) % 4 + 2, :],
                            )
                    elif pr % 2 == 1:
                        for m2 in range(2):
                            nc.sync.dma_start(
                                out=o_dram[n, m2 * 128:(m2 + 1) * 128,
                                           hb * 4:(hb + 1) * 4, :],
                                in_=out_half[m2][hb][:],
                            )

    _split_sync_waits(nc, max_waits=1)
    return nc


_CACHE = {}


def _ensure_neuron_backend():
    """If the caller pinned JAX_PLATFORMS=cpu (common for running the jax
    reference), re-open the accelerator platform for the bass run."""
    import jax

    try:
        if any(d.platform != "cpu" for d in jax.devices()):
            return
    except Exception:
        pass
    os.environ["JAX_PLATFORMS"] = ""
    try:
        from jax._src import xla_bridge

        xla_bridge._clear_backends()
        jax.devices()
    except Exception:
        pass



def _install_trace_shims():
    """Best-effort: let run_bass_kernel_spmd(trace=True) work in this
    container -- antenv.axon_hooks is missing from the image and the
    artifact bucket is unreachable; degrade both gracefully."""
    try:
        import antenv.axon_hooks  # noqa: F401
    except ImportError:
        import types

        hook = None
        try:
            if "/root/.axon_site" not in sys.path and os.path.isdir("/root/.axon_site"):
                sys.path.insert(0, "/root/.axon_site")
            from trn_agent_boot.trn_boot import _ntff_profile_via_ctypes

            hook = _ntff_profile_via_ctypes("/opt/axon/libaxon_pjrt.so")
        except Exception:
            hook = None
        mod = types.ModuleType("antenv.axon_hooks")
        mod.get_axon_ntff_profile_hook = lambda: hook
        mod.set_axon_ntff_profile_hook = lambda h: None
        sys.modules["antenv.axon_hooks"] = mod
    try:
        import concourse.bass_utils as bu

        if not getattr(bu.upload_artifacts, "_safe", False):
            _orig = bu.upload_artifacts

            def _safe_upload(tmpdir):
                try:
                    return _orig(tmpdir)
                except Exception:
                    return tmpdir

            _safe_upload._safe = True
            bu.upload_artifacts = _safe_upload
    except Exception:
        pass


def kernel(x: np.ndarray) -> np.ndarray:
    x = np.asarray(x)
    assert x.shape == (N_IMG, SIZE, SIZE, 1) and x.dtype == np.float32
    _ensure_neuron_backend()
    _install_trace_shims()
    if "nc" not in _CACHE:
        if ALGO == "g":
            _CACHE["nc"] = _build_program_g()
            _CACHE["g"], _CACHE["gt"] = _build_g_matrices()
        elif ALGO == "batch":
            _CACHE["nc"] = _build_program_batch()
            _CACHE["d"] = _build_d_matrices()
        elif ALGO == "fold":
            _CACHE["nc"] = _build_program_fold()
            _CACHE["d"] = _build_fold_d()
        else:
            _CACHE["nc"] = _build_program_direct()
            _CACHE["d"] = _build_d_matrices()
    nc = _CACHE["nc"]

    xs = x.reshape(N_IMG, SIZE, SIZE)
    if ALGO == "fold":
        in_maps = [
            {"x": _fold_x(xs[c * PER_CORE:(c + 1) * PER_CORE]), "d": _CACHE["d"]}
            for c in range(N_CORES)
        ]
        res = run_bass_kernel_spmd(nc, in_maps, list(range(N_CORES)))
        kernel._last_results = res
        out = np.empty((N_IMG, SIZE, SIZE, N_U + 1), np.float32)
        out[..., 0] = xs
        # channel 8 (u=127): D = I - n n^T with n the Nyquist vector; exact.
        nv = (((-1) ** np.arange(SIZE)) / 16.0).astype(np.float32)
        v = np.einsum("p,ipq->iq", nv, xs)          # n^T X   [img, 256]
        w = np.einsum("ipq,q->ip", xs, nv)          # X n     [img, 256]
        s = v @ nv                                   # n^T X n [img]
        out[..., 8] = (xs - nv[None, :, None] * v[:, None, :]
                       - w[:, :, None] * nv[None, None, :]
                       + s[:, None, None] * np.outer(nv, nv)[None])
        # device channels 1..7: unfold parity quadrants (outT on device)
        # flat pair stream: chunk ci = seg*7+u, (b, half) by segment; decode
        # into [n_ch, 2(b), 128(sb), img, 2(a), sa]
        seg_bh = ((0, 0), (1, 0), (0, 1), (1, 1))
        def _core_arr(c):
            raw = np.asarray(res.results[c]["out"]).astype(np.float32)
            raw = raw.reshape(N_UD * 2, 128, 2, PER_CORE // 2, 2, 128)
            a = np.empty((N_UD, 2, 128, PER_CORE, 2, 128), np.float32)
            for ci in range(4 * N_UD):
                seg, u = divmod(ci, N_UD)
                b, half = seg_bh[seg]
                a[u, b, :, half * 4:(half + 1) * 4] = raw[ci // 2, :, ci % 2]
            return a
        arr = np.concatenate([_core_arr(c) for c in range(N_CORES)], axis=3)
        for ui in range(N_UD):
            acc = np.zeros((N_IMG, SIZE, SIZE), np.float32)
            for a, sa in ((0, 1.0), (1, -1.0)):
                for b, sb in ((0, 1.0), (1, -1.0)):
                    # [sb, img, sa] -> [img, sa, sb]
                    q = arr[ui, b, :, :, a, :].astype(np.float32).transpose(1, 2, 0)
                    acc[:, :128, :128] += q
                    acc[:, 128:, :128] += sa * q[:, ::-1, :]
                    acc[:, :128, 128:] += sb * q[:, :, ::-1]
                    acc[:, 128:, 128:] += (sa * sb) * q[:, ::-1, ::-1]
            out[..., ui + 1] = acc
        return out

    extras = {"d": _CACHE["d"]} if ALGO != "g" else {"g": _CACHE["g"], "gt": _CACHE["gt"]}
    in_maps = [
        {"x": np.ascontiguousarray(xs[c * PER_CORE:(c + 1) * PER_CORE]), **extras}
        for c in range(N_CORES)
    ]
    res = run_bass_kernel_spmd(nc, in_maps, list(range(N_CORES)))
    kernel._last_results = res
    out = np.empty((N_IMG, SIZE, SIZE, N_U + 1), np.float32)
    out[..., 0] = xs
    filt = np.concatenate([res.results[c]["out"] for c in range(N_CORES)], axis=0)
    if ALGO == "batch":
        # device output is [n, h, c, w] (transposed); reorder to [n, w, h, c]
        out[..., 1:] = filt.transpose(0, 3, 1, 2)
    else:
        # device output is [n, w, c, h] planar; reorder to [n, w, h, c]
        out[..., 1:] = filt.transpose(0, 1, 3, 2)
    return out



# revision 60
# speedup vs baseline: 1.0401x; 1.0401x over previous
"""Trainium2 Bass kernel for nn_BlurLayer: batched FFT2D low-pass filter bank.

Math: for each 256x256 image X, each cutoff u, the reference computes
Re(IFFT2(ifftshift(mask_u * fftshift(FFT2(X))))) with mask_u a centered
(2u+1)^2 block of ones.  That ideal low-pass filter is separable and equals
D_u @ X @ D_u with the real symmetric projection D_u = G_u G_u^T, where the
columns of G are the orthonormal real Fourier vectors ordered by |frequency|
(DC, cos1, sin1, ..., cos127, sin127, 0-pad), and G_u = G[:, :2u+1].

The kernel evaluates, per image, the shared spectral analysis

    out1 = matmul(lhsT=X,    rhs=G)    # X^T G            (stage A)
    S    = matmul(lhsT=out1, rhs=G)    # G^T X G          (stage B)

and per cutoff u the rank-(2u+1) synthesis (r = 2u+1)

    out3 = matmul(lhsT=S[:r,:r], rhs=G^T[:r])   # S_u^T G_u^T   (stage C)
    out  = matmul(lhsT=out3,     rhs=G^T[:r])   # G_u S_u G_u^T (stage D)

All four are plain TensorE matmuls chained through the stationary operand
(out = lhsT.T @ rhs), so no transposes are needed and the result lands as
[w, h], exactly the output layout.  Data parallel over 8 NeuronCores, 8
images per core.  The device writes the 8 filtered channels planar
([n, w, c, h]); channel interleave and the channel-0 passthrough are
assembled on the host.
"""

import os
import sys

import numpy as np

for _p in ("/opt/trn_rl_repo", "/root/.axon_site/_ro/trn_rl_repo"):
    if os.path.isdir(_p) and _p not in sys.path:
        sys.path.insert(0, _p)

import concourse.bass as bass
import concourse.mybir as mybir
from concourse.tile import TileContext
from concourse.bass_utils import run_bass_kernel_spmd

# ---------------------------------------------------------------- problem spec
SIZE = 256
N_IMG = 64
N_CORES = 8
PER_CORE = N_IMG // N_CORES  # 8 images per core
US = [int(v) for v in np.linspace(5.0, SIZE // 2 - 1.0, 8)]  # [5,...,127]
N_U = len(US)
N_PAIR = N_U // 2

# matmul operand dtype: "f32r" (full-rate fp32 mode), "f32" (exact, 4x slower)
MM_MODE = os.environ.get("BLUR_MM_MODE", "f32r")
ALGO = os.environ.get("BLUR_ALGO", "fold")  # "fold", "direct", "batch", or "g"

_F32 = mybir.dt.float32
_F32R = mybir.dt.float32r
_F16 = mybir.dt.float16
_MM_DT = _F32R if MM_MODE == "f32r" else _F32

# ---- folded-parity algorithm ("fold") constants -----------------------------
# Channels 1..7 (u in US[:7]) run on device; channel 8 (u=127) is I minus the
# Nyquist rank-1 projector, reconstructed exactly on the host; channel 0 is the
# passthrough.  Each 256x256 image folds under the reflection p <-> 255-p into
# 4 parity quadrants of exactly 128x128 (the half-sample cos/sin eigenbasis of
# any symmetric circulant filter splits 128/128 with no fixed points), and
# D_u X D_u decomposes into independent per-quadrant products
#     out_ab = De/o_u @ X_ab @ De/o_u            (a, b = row/col parity)
# with folded filters  Dpar_u = 0.5 * (k(p-q) +/- k(p+q+1)),  k = Dirichlet.
# The 0.5 per side absorbs the 1/4 of the two-sided unfold.
N_UD = 7  # device channels
US_DEV = US[:N_UD]
# channels shipped as fp8e4m3 instead of fp16 (disabled: the ~1us DMA saving
# is not worth shrinking the 33x error margin to 3x)
N_F8 = 0


def _build_g_matrices():
    """G [space, col] with cols (DC, cos1, sin1, ..., cos127, sin127, 0).
    Returns (g, gt): g[k, p, c] = G[128k+p, c]; gt[k, p, c] = G^T[128k+p, c]."""
    a = np.arange(SIZE)
    cols = [np.full(SIZE, 1.0 / np.sqrt(SIZE))]
    for f in range(1, 128):
        cols.append(np.sqrt(2.0 / SIZE) * np.cos(2 * np.pi * f * a / SIZE))
        cols.append(np.sqrt(2.0 / SIZE) * np.sin(2 * np.pi * f * a / SIZE))
    cols.append(np.zeros(SIZE))
    G = np.stack(cols, axis=1).astype(np.float32)
    g = np.stack([G[:128], G[128:]])
    GT = np.ascontiguousarray(G.T)
    gt = np.stack([GT[:128], GT[128:]])
    return g, gt


def _build_d_matrices() -> np.ndarray:
    """d[p, u*256+j] = D_u[p, j] (top half only; the bottom half is the
    128-column rotation, derived on device), float32, [128, 8*256]."""
    a = np.arange(SIZE)
    diff = a[:, None] - a[None, :]
    d = np.empty((128, N_U * SIZE), np.float32)
    for ui, u in enumerate(US):
        f = np.arange(1, u + 1)
        acc = np.ones((SIZE, SIZE), np.float64)
        ang = 2.0 * np.pi * diff[..., None] * f / SIZE
        acc += 2.0 * np.cos(ang).sum(axis=-1)
        Du = (acc / SIZE).astype(np.float32)
        d[:, ui * SIZE:(ui + 1) * SIZE] = Du[:128]
    return d


def _dirichlet(t: np.ndarray, u: int) -> np.ndarray:
    """Closed-form symmetric ideal low-pass kernel k_u(t), period 256."""
    t = np.asarray(t, np.float64)
    s = np.sin(np.pi * t / SIZE)
    with np.errstate(divide="ignore", invalid="ignore"):
        k = np.sin(np.pi * (2 * u + 1) * t / SIZE) / (SIZE * s)
    return np.where(np.abs(s) < 1e-12, (2 * u + 1) / SIZE, k)


def _build_fold_d() -> np.ndarray:
    """Folded filter banks, 0.5-scaled: d[par, p, u*128 + q], fp16 [2,128,896]."""
    p = np.arange(128)
    diff = p[:, None] - p[None, :]
    ssum = p[:, None] + p[None, :] + 1
    d = np.empty((2, 128, N_UD * 128), np.float16)
    for ui, u in enumerate(US_DEV):
        kd = _dirichlet(diff, u)
        ks = _dirichlet(ssum, u)
        d[0, :, ui * 128:(ui + 1) * 128] = (0.5 * (kd + ks)).astype(np.float16)
        d[1, :, ui * 128:(ui + 1) * 128] = (0.5 * (kd - ks)).astype(np.float16)
    return d


def _fold_x(xs: np.ndarray) -> np.ndarray:
    """Fold a core's images [8,256,256] f32 into quadrants.
    Returns [2, 128, 8*2*128] fp16 laid out [a, wa, (img, b, hb)]."""
    xr = [xs[:, :128, :] + xs[:, ::-1, :][:, :128, :],
          xs[:, :128, :] - xs[:, ::-1, :][:, :128, :]]
    out = np.empty((2, 128, PER_CORE, 2, 128), np.float16)
    for a in range(2):
        for b in range(2):
            sb = 1.0 if b == 0 else -1.0
            q = xr[a][:, :, :128] + sb * xr[a][:, :, ::-1][:, :, :128]
            out[a, :, :, b, :] = q.transpose(1, 0, 2)
    return out.reshape(2, 128, PER_CORE * 2 * 128)


def _build_program_fold() -> bass.Bass:
    """Folded-parity fp16 pipeline.

    stage 1 (per img, a, b):  M1 = X_ab^T @ D_ua     [hb, space-a]
    stage 2 (per u, b):       outT = D_ub @ M1       [space-b, (img, a, space-a)]
    lhsT is always a [128,128] fp16 stationary (FWL-eligible); stage-2 streams
    512-col chunks of many images per weight load.  PSUM is drained by vector
    and scalar alternately (the binding resource); output leaves as fp16."""
    nc = bass.Bass()
    _F8 = mybir.dt.float8e4
    x_dram = nc.declare_dram_parameter("x", [2, 128, PER_CORE * 2 * 128], _F16, isOutput=False)
    d_dram = nc.declare_dram_parameter("d", [2, 128, N_UD * 128], _F16, isOutput=False)
    o_dram = nc.declare_dram_parameter(
        "out", [N_UD - N_F8, 2, 128, PER_CORE * 2 * 128], _F16, isOutput=True)
    o8_dram = (nc.declare_dram_parameter(
        "out8", [N_F8, 2, 128, PER_CORE * 2 * 128], _F8, isOutput=True)
        if N_F8 else None)

    with TileContext(nc) as tc:
        with (
            tc.tile_pool(name="xin", bufs=1) as xin_pool,
            tc.tile_pool(name="dmat", bufs=1) as d_pool,
            tc.tile_pool(name="m1", bufs=1) as m1_pool,
            tc.tile_pool(name="ot", bufs=20) as ot_pool,
            tc.tile_pool(name="scr", bufs=1) as scr_pool,
            tc.tile_pool(name="ps", bufs=4, space="PSUM") as ps_pool,
        ):
            # scratch for PE warm-up (memset first so warm-up can start early)
            scr = scr_pool.tile([128, 512], _F16, tag="scr", name="scr")
            nc.vector.memset(scr[:], 0.0)

            # input DMAs: one per engine ring so each is first in its ring and
            # descriptor generation runs in parallel; tensor/vector rings come
            # out of the engine prologue earliest.
            xa = [None, None]
            for a in range(2):
                xa[a] = xin_pool.tile([128, PER_CORE * 2 * 128], _F16,
                                      tag=f"x{a}", name=f"x_{a}")
            dt = [None, None]
            for par in range(2):
                dt[par] = d_pool.tile([128, N_UD * 128], _F16, tag=f"d{par}",
                                      name=f"d_{par}")
            # d on the otherwise-idle scalar ring; x quartered so the first
            # stage-1 blocks can start as soon as imgs 0-1 land
            nc.scalar.dma_start(out=dt[0][:], in_=d_dram[0])
            nc.scalar.dma_start(out=dt[1][:], in_=d_dram[1])
            nc.sync.dma_start(out=xa[0][:, 0:512], in_=x_dram[0][:, 0:512])
            nc.gpsimd.dma_start(out=xa[1][:, 0:512], in_=x_dram[1][:, 0:512])
            nc.sync.dma_start(out=xa[0][:, 512:1024], in_=x_dram[0][:, 512:1024])
            nc.gpsimd.dma_start(out=xa[1][:, 512:1024], in_=x_dram[1][:, 512:1024])
            nc.sync.dma_start(out=xa[0][:, 1024:2048], in_=x_dram[0][:, 1024:2048])
            nc.gpsimd.dma_start(out=xa[1][:, 1024:2048], in_=x_dram[1][:, 1024:2048])
            m1 = [None, None]
            for b in range(2):
                m1[b] = m1_pool.tile([128, N_UD, PER_CORE * 2 * 128], _F16,
                                     tag=f"m1{b}", name=f"m1_{b}")

            # PE warm-up: HAM un-throttles after ~3.4us of sustained matmul
            # activity; burn the input-DMA wait on dummy matmuls over the
            # memset scratch tile so stage 1 starts at 2.4 GHz.
            s0 = ps_pool.tile([128, 1024], _F32, tag="ps", name="s_warm")
            for wi in range(11):
                nc.tensor.matmul(s0[:, 0:512], scr[:, 0:128], scr[:],
                                 start=True, stop=True, skip_group_check=True)

            drains = [0]

            def drain(dst, src):
                if drains[0] % 2 == 0:
                    nc.scalar.copy(dst, src)
                else:
                    nc.vector.tensor_copy(dst, src)
                drains[0] += 1

            def s1_block(img, a, b):
                s1 = ps_pool.tile([128, 1024], _F32, tag="ps",
                                  name=f"s1_{img}_{a}_{b}")
                lhsT = xa[a][:, (img * 2 + b) * 128:(img * 2 + b + 1) * 128]
                nc.tensor.matmul(s1[:, 0:512], lhsT, dt[a][:, 0:512],
                                 start=True, stop=True, skip_group_check=True)
                nc.tensor.matmul(s1[:, 512:N_UD * 128], lhsT,
                                 dt[a][:, 512:N_UD * 128],
                                 start=True, stop=True, skip_group_check=True)
                dst = m1[b].rearrange("p u (i c) -> p u i c", c=128)[
                    :, :, img * 2 + a, :]
                src = s1[:, 0:N_UD * 128].rearrange("p (u c) -> p u c", c=128)
                drain(dst, src)

            rings = [0]

            def s2_chunk(u, b, half, q=None):
                f8 = u < N_F8
                lhsT = dt[b][:, u * 128:(u + 1) * 128]
                k2s = range(2) if q is None else (q,)
                w = 512 * len(k2s)
                s2 = ps_pool.tile([128, 1024], _F32, tag="ps",
                                  name=f"s2_{half}_{u}_{b}_{q}")
                for i, k2 in enumerate(k2s):
                    rhs = m1[b][:, u, 1024 * half + 512 * k2:
                                1024 * half + 512 * (k2 + 1)]
                    nc.tensor.matmul(s2[:, 512 * i:512 * (i + 1)], lhsT, rhs,
                                     start=True, stop=True, skip_group_check=True)
                ot = ot_pool.tile([128, w], _F8 if f8 else _F16, tag="ot",
                                  name=f"ot_{half}_{u}_{b}_{q}")
                drain(ot[:], s2[:, 0:w])
                dst = (o8_dram[u] if f8 else o_dram[u - N_F8])[b]
                off = 1024 * half + 512 * (q or 0)
                if rings[0] >= 25 and w == 1024:
                    # final chunks: halve transfer latency by splitting each
                    # across two rings in parallel
                    nc.sync.dma_start(out=dst[:, off:off + 512], in_=ot[:, 0:512])
                    nc.gpsimd.dma_start(out=dst[:, off + 512:off + 1024],
                                        in_=ot[:, 512:1024])
                else:
                    dma_eng = (nc.sync, nc.gpsimd, nc.scalar, nc.sync, nc.gpsimd)[
                        rings[0] % 5]
                    dma_eng.dma_start(out=dst[:, off:off + w], in_=ot[:])
                rings[0] += 1

            def interleave(s1_args, s2_args):
                s1_it, s2_it = iter(s1_args), iter(s2_args)
                while True:
                    done = 0
                    for it, fn in ((s1_it, s1_block), (s2_it, s2_chunk)):
                        try:
                            fn(*next(it))
                        except StopIteration:
                            done += 1
                    if done == 2:
                        break

            # b-split schedule: stage-2 output production starts after only 8
            # stage-1 blocks and stays roughly uniform, so the output DMA
            # rings (the end-to-end critical path) run from ~18us onward.
            h0, h1 = (0, 1, 2, 3), (4, 5, 6, 7)
            interleave([(i, a, 0) for i in h0 for a in range(2)], [])
            interleave([(i, a, 1) for i in h0 for a in range(2)],
                       [(u, 0, 0) for u in range(N_UD)])
            interleave([(i, a, 0) for i in h1 for a in range(2)],
                       [(u, 1, 0) for u in range(N_UD)])
            interleave([(i, a, 1) for i in h1 for a in range(2)],
                       [(u, 0, 1) for u in range(N_UD)])
            interleave([], [(u, 1, 1) for u in range(N_UD)])

    _split_sync_waits(nc, max_waits=1)
    return nc


def _split_sync_waits(nc, max_waits=1):
    """Walrus in this container only accepts 1 sync-wait per instruction;
    hoist excess waits onto same-engine NOPs inserted just before."""
    for f in nc.m.functions:
        for bb in f.blocks:
            insts = bb.instructions
            i = 0
            while i < len(insts):
                inst = insts[i]
                si = inst.sync_info
                if si is not None and si.on_wait and len(si.on_wait) > max_waits:
                    waits = list(si.on_wait)
                    keep = waits[-max_waits:]
                    excess = waits[:-max_waits]
                    si.on_wait = keep
                    eng = nc.engines[inst.engine]
                    new_nops = []
                    for j in range(0, len(excess), max_waits):
                        chunk = excess[j:j + max_waits]
                        nop = eng.nop(nofuse=True, hint=f"wsplit_{inst.name}_{j}")
                        nop_inst = nop.ins if hasattr(nop, "ins") else nop
                        for f2 in nc.m.functions:
                            for bb2 in f2.blocks:
                                if nop_inst in bb2.instructions and not (
                                    bb2 is bb and bb2.instructions.index(nop_inst) < i
                                ):
                                    bb2.instructions.remove(nop_inst)
                        if nop_inst.sync_info is None:
                            nop_inst.sync_info = mybir.SyncInfo(
                                on_wait=chunk, on_update=[]
                            )
                        else:
                            nop_inst.sync_info.on_wait = chunk
                        new_nops.append(nop_inst)
                    for k, nop_inst in enumerate(new_nops):
                        insts.insert(i + k, nop_inst)
                    i += len(new_nops)
                i += 1


def _strip_redundant_mm_incs(nc):
    """Drop then_inc updates on matmuls where they are provably unobserved.
    Serialized sem increments cost ~26ns each on the PE.  An increment is
    kept iff it belongs to a stop matmul OR some wait references its exact
    cumulative value (this includes the same-engine PSUM-WAR guards that
    deadlocked the naive round-up version).  All awaited values then map
    exactly onto retained increments, so no wait can move past its original
    producer."""
    import concourse.mybir as mb

    pe_sem_id = None
    inc_events = []
    for f in nc.m.functions:
        for bb in f.blocks:
            for inst in bb.instructions:
                si = inst.sync_info
                if not (isinstance(inst, mb.InstMatmult) and si and si.on_update):
                    continue
                for upd in si.on_update:
                    uid = getattr(upd, "id", None)
                    if pe_sem_id is None:
                        pe_sem_id = uid
                    if uid == pe_sem_id:
                        inc_events.append((inst, upd))
    if pe_sem_id is None:
        return 0
    # all waits on this sem; abort on anything but simple sem-ge-imm
    awaited = set()
    for f in nc.m.functions:
        for bb in f.blocks:
            for inst in bb.instructions:
                si = inst.sync_info
                if si and si.on_wait:
                    for w in si.on_wait:
                        if getattr(w, "id", None) == pe_sem_id:
                            if w.wait_mode != "sem-ge-imm" or w.wait_reg is not None:
                                return 0
                            awaited.add(w.wait_value)
    keep_flags = []
    for v, (inst, _upd) in enumerate(inc_events, start=1):
        keep_flags.append(bool(inst.stop_tensor_calc) or v in awaited)
    new_of_old = {}
    kept = 0
    for v, k in enumerate(keep_flags, start=1):
        if k:
            kept += 1
        new_of_old[v] = kept
    if any(v not in new_of_old or not keep_flags[v - 1] for v in awaited):
        return 0  # paranoia: every awaited value must be a retained inc
    for f in nc.m.functions:
        for bb in f.blocks:
            for inst in bb.instructions:
                si = inst.sync_info
                if si and si.on_wait:
                    for w in si.on_wait:
                        if getattr(w, "id", None) == pe_sem_id:
                            w.wait_value = new_of_old[w.wait_value]
    n_dropped = 0
    for (inst, upd), k in zip(inc_events, keep_flags):
        if not k:
            inst.sync_info.on_update = [
                u for u in inst.sync_info.on_update if u is not upd
            ]
            n_dropped += 1
    return n_dropped


def _build_program_g() -> bass.Bass:
    nc = bass.Bass()
    x_dram = nc.declare_dram_parameter("x", [PER_CORE, SIZE, SIZE], _MM_DT, isOutput=False)
    g_dram = nc.declare_dram_parameter("g", [2, 128, SIZE], _MM_DT, isOutput=False)
    gt_dram = nc.declare_dram_parameter("gt", [2, 128, SIZE], _MM_DT, isOutput=False)
    # planar channel layout [n, w, c, h]; host reorders to [n, w, h, c]
    o_dram = nc.declare_dram_parameter("out", [PER_CORE, SIZE, N_U, SIZE], _F32, isOutput=True)

    with TileContext(nc) as tc:
        with (
            tc.tile_pool(name="xin", bufs=2 * PER_CORE) as xin_pool,
            tc.tile_pool(name="gmat", bufs=4) as g_pool,
            tc.tile_pool(name="oA", bufs=2) as oA_pool,
            tc.tile_pool(name="oS", bufs=2) as oS_pool,
            tc.tile_pool(name="oC", bufs=6) as oC_pool,
            tc.tile_pool(name="obig", bufs=4) as obig_pool,
            tc.tile_pool(name="psAB", bufs=3, space="PSUM") as psAB_pool,
            tc.tile_pool(name="psC", bufs=2, space="PSUM") as psC_pool,
            tc.tile_pool(name="psD", bufs=3, space="PSUM") as psD_pool,
        ):
            # G/GT tiles first (small, gate the first matmuls)
            g_t, gt_t = [None, None], [None, None]
            for k in range(2):
                g_t[k] = g_pool.tile([128, SIZE], _MM_DT, tag="g", name=f"g_{k}")
                nc.sync.dma_start(out=g_t[k][:], in_=g_dram[k])
            for k in range(2):
                gt_t[k] = g_pool.tile([128, SIZE], _MM_DT, tag="gt", name=f"gt_{k}")
                nc.sync.dma_start(out=gt_t[k][:], in_=gt_dram[k])

            # X tiles on the gpsimd (SWDGE) queue so they don't serialize
            # behind output DMAs on the sync queue
            x_t = [[None] * PER_CORE for _ in range(2)]
            for n in range(PER_CORE):
                for k in range(2):
                    t = xin_pool.tile([128, SIZE], _MM_DT, tag=f"x{k}", name=f"x_{k}_{n}")
                    nc.gpsimd.dma_start(out=t[:], in_=x_dram[n, k * 128:(k + 1) * 128, :])
                    x_t[k][n] = t

            for n in range(PER_CORE):
                # ---- stage A: out1 = X^T G, h-blocks in free halves
                sA = psAB_pool.tile([128, 512], _F32, tag="sAB", name=f"sA_{n}")
                for m in range(2):
                    for k in range(2):
                        nc.tensor.matmul(
                            sA[:, m * 256:(m + 1) * 256],
                            x_t[k][n][:, m * 128:(m + 1) * 128],
                            g_t[k][:],
                            start=(k == 0),
                            stop=(k == 1),
                            skip_group_check=True,
                        )
                oA = oA_pool.tile([128, 512], _MM_DT, tag="oA", name=f"oA_{n}")
                nc.vector.tensor_copy(oA[:], sA[:])

                # ---- stage B: S = G^T X G, f1-blocks in free halves
                sB = psAB_pool.tile([128, 512], _F32, tag="sAB", name=f"sB_{n}")
                for mB in range(2):
                    for kB in range(2):
                        nc.tensor.matmul(
                            sB[:, mB * 256:(mB + 1) * 256],
                            oA[:, kB * 256 + mB * 128: kB * 256 + (mB + 1) * 128],
                            g_t[kB][:],
                            start=(kB == 0),
                            stop=(kB == 1),
                            skip_group_check=True,
                        )
                oS = oS_pool.tile([128, 512], _MM_DT, tag="oS", name=f"oS_{n}")
                nc.scalar.copy(oS[:], sB[:])

                # ---- stages C+D per pair of cutoffs
                out_big = [
                    obig_pool.tile([128, N_U, SIZE], _F32, tag="ob", name=f"ob_{n}_{m2b}")
                    for m2b in range(2)
                ]
                for pr in range(N_PAIR):
                    oC = [None, None]
                    for ha in range(2):
                        u = US[2 * pr + ha]
                        r = 2 * u + 1
                        nblk = 1 if r <= 128 else 2
                        sC = psC_pool.tile([128, 512], _F32, tag="sC", name=f"sC_{n}_{pr}_{ha}")
                        for m3 in range(nblk):
                            m3w = min(128, r - m3 * 128)
                            for c1 in range(nblk):
                                c1w = min(128, r - c1 * 128)
                                nc.tensor.matmul(
                                    sC[0:m3w, m3 * 256:m3 * 256 + 256],
                                    oS[0:c1w, c1 * 256 + m3 * 128: c1 * 256 + m3 * 128 + m3w],
                                    gt_t[c1][0:c1w, :],
                                    start=(c1 == 0),
                                    stop=(c1 == nblk - 1),
                                    skip_group_check=True,
                                )
                        oCt = oC_pool.tile([128, 512], _MM_DT, tag="oC", name=f"oC_{n}_{pr}_{ha}")
                        if ha == 0:
                            nc.vector.tensor_copy(oCt[:, 0:256 * nblk], sC[:, 0:256 * nblk])
                        else:
                            nc.scalar.copy(oCt[:, 0:256 * nblk], sC[:, 0:256 * nblk])
                        oC[ha] = oCt

                    for m2 in range(2):
                        sD = psD_pool.tile([128, 2, SIZE], _F32, tag="sD", name=f"sD_{n}_{pr}_{m2}")
                        for ha in range(2):
                            u = US[2 * pr + ha]
                            r = 2 * u + 1
                            nkD = 1 if r <= 128 else 2
                            for kD in range(nkD):
                                kw = min(128, r - kD * 128)
                                nc.tensor.matmul(
                                    sD[:, ha, :],
                                    oC[ha][0:kw, kD * 256 + m2 * 128: kD * 256 + m2 * 128 + 128],
                                    gt_t[kD][0:kw, :],
                                    start=(kD == 0),
                                    stop=(kD == nkD - 1),
                                    skip_group_check=True,
                                )
                        dst = out_big[m2][:, 2 * pr:2 * pr + 2, :]
                        if m2 == 0:
                            nc.vector.tensor_copy(dst, sD[:])
                        else:
                            nc.scalar.copy(dst, sD[:])

                for m2 in range(2):
                    nc.sync.dma_start(
                        out=o_dram[n, m2 * 128:(m2 + 1) * 128, :, :],
                        in_=out_big[m2][:],
                    )

    _split_sync_waits(nc, max_waits=1)
    return nc


def _build_program_batch() -> bass.Bass:
    """Direct algorithm with stage-2 flipped: D_u stationary, o1 moving with
    TWO images batched per rhs (N=512 everywhere, 32 MMs/image instead of 48).
    Stage-2 output comes out transposed (out_u^T), so the device writes
    [n, h, c, w] planar and the host transposes to [n, w, h, c]."""
    nc = bass.Bass()
    x_dram = nc.declare_dram_parameter("x", [PER_CORE, SIZE, SIZE], _MM_DT, isOutput=False)
    d_dram = nc.declare_dram_parameter("d", [128, N_U * SIZE], _MM_DT, isOutput=False)
    o_dram = nc.declare_dram_parameter("out", [PER_CORE, SIZE, N_U, SIZE], _F32, isOutput=True)

    with TileContext(nc) as tc:
        with (
            tc.tile_pool(name="xin", bufs=2 * PER_CORE) as xin_pool,
            tc.tile_pool(name="dmat", bufs=2 * N_PAIR) as d_pool,
            tc.tile_pool(name="o1", bufs=6) as o1_pool,
            tc.tile_pool(name="obig", bufs=6) as obig_pool,
            tc.tile_pool(name="ps1", bufs=2, space="PSUM") as ps1_pool,
            tc.tile_pool(name="ps2", bufs=4, space="PSUM") as ps2_pool,
        ):
            d_t = [[None] * N_PAIR for _ in range(2)]
            x_t = [[None] * PER_CORE for _ in range(2)]

            def load_d(k, pr):
                if k == 0:
                    t = d_pool.tile([128, 512], _MM_DT, tag="d0", name=f"d_0_{pr}")
                    nc.sync.dma_start(out=t[:], in_=d_dram[:, pr * 512:(pr + 1) * 512])
                    d_t[0][pr] = t
                else:
                    t = d_pool.tile([128, 512], _MM_DT, tag="d1", name=f"d_1_{pr}")
                    d0 = d_t[0][pr]
                    for ha in range(2):
                        b = ha * 256
                        nc.vector.tensor_copy(
                            t[:, b:b + 128], d0[:, b + 128:b + 256].bitcast(_F32))
                        nc.vector.tensor_copy(
                            t[:, b + 128:b + 256], d0[:, b:b + 128].bitcast(_F32))
                    d_t[1][pr] = t

            def load_x(k, n, eng):
                t = xin_pool.tile([128, SIZE], _MM_DT, tag=f"x{k}", name=f"x_{k}_{n}")
                eng.dma_start(out=t[:], in_=x_dram[n, k * 128:(k + 1) * 128, :])
                x_t[k][n] = t

            load_d(0, 0)
            load_x(0, 0, nc.gpsimd)
            load_d(1, 0)
            load_x(1, 0, nc.gpsimd)
            for pr in range(1, N_PAIR):
                load_d(0, pr)
                load_d(1, pr)
            for n in range(1, PER_CORE):
                load_x(0, n, nc.gpsimd)
                load_x(1, n, nc.gpsimd)

            for ip in range(PER_CORE // 2):
                nA, nB = 2 * ip, 2 * ip + 1
                # ---- stage 1: o1g[p, kp*1024 + img*512 + paircol]
                #      = (X_img^T D_pair)[kp*128+p, paircol]
                o1g = [None] * N_PAIR
                for pr in range(N_PAIR):
                    o1gt = o1_pool.tile([128, 2048], _MM_DT, tag="o1", name=f"o1_{ip}_{pr}")
                    for kp in range(2):
                        s1 = ps1_pool.tile([128, 1024], _F32, tag="s1", name=f"s1_{ip}_{pr}_{kp}")
                        for img, n in enumerate((nA, nB)):
                            for k in range(2):
                                nc.tensor.matmul(
                                    s1[:, img * 512:(img + 1) * 512],
                                    x_t[k][n][:, kp * 128:(kp + 1) * 128],
                                    d_t[k][pr][:],
                                    start=(k == 0),
                                    stop=(k == 1),
                                    skip_group_check=True,
                                )
                        dst = o1gt[:, kp * 1024:(kp + 1) * 1024]
                        if (pr + kp) % 2 == 0:
                            nc.vector.tensor_copy(dst, s1[:])
                        else:
                            nc.scalar.copy(dst, s1[:])
                    o1g[pr] = o1gt

                # ---- stage 2: D stationary, both images moving (N=512)
                # psum = out_u^T blocks: [mj(part) = h-axis, (img, w)]
                ob = [
                    [
                        obig_pool.tile([128, 2, N_U // 2, SIZE], _F32, tag="ob",
                                       name=f"ob_{ip}_{m}_{hb}")
                        for hb in range(2)
                    ]
                    for m in range(2)
                ]
                for pr in range(N_PAIR):
                    hb = pr // 2
                    for ha in range(2):
                        ci = (2 * pr + ha) % 4
                        for m in range(2):
                            s2 = ps2_pool.tile([128, 2, 256], _F32, tag="s2",
                                               name=f"s2_{ip}_{pr}_{ha}_{m}")
                            o1v = o1g[pr].rearrange("p (a b c) -> p a b c", a=2, b=2, c=512)
                            for kp in range(2):
                                lhsT = d_t[kp][pr][:, ha * 256 + m * 128:
                                                   ha * 256 + (m + 1) * 128]
                                rhs = o1v[:, kp, :, ha * 256:(ha + 1) * 256]
                                nc.tensor.matmul(
                                    s2[:],
                                    lhsT,
                                    rhs,
                                    start=(kp == 0),
                                    stop=(kp == 1),
                                    skip_group_check=True,
                                )
                            dst = ob[m][hb][:, :, ci, :]
                            if m == 0:
                                nc.vector.tensor_copy(dst, s2[:])
                            else:
                                nc.scalar.copy(dst, s2[:])
                    if pr % 2 == 1:
                        for m in range(2):
                            for img, n in enumerate((nA, nB)):
                                nc.sync.dma_start(
                                    out=o_dram[n, m * 128:(m + 1) * 128,
                                               hb * 4:(hb + 1) * 4, :],
                                    in_=ob[m][hb][:, img, :, :],
                                )

    _split_sync_waits(nc, max_waits=1)
    return nc


def _build_program_direct() -> bass.Bass:
    nc = bass.Bass()
    x_dram = nc.declare_dram_parameter("x", [PER_CORE, SIZE, SIZE], _MM_DT, isOutput=False)
    d_dram = nc.declare_dram_parameter("d", [128, N_U * SIZE], _MM_DT, isOutput=False)
    o_dram = nc.declare_dram_parameter("out", [PER_CORE, SIZE, N_U, SIZE], _F32, isOutput=True)

    with TileContext(nc) as tc:
        with (
            tc.tile_pool(name="xin", bufs=2 * PER_CORE) as xin_pool,
            tc.tile_pool(name="dmat", bufs=2 * N_PAIR) as d_pool,
            tc.tile_pool(name="o1", bufs=2 * N_PAIR) as o1_pool,
            tc.tile_pool(name="obig", bufs=8) as obig_pool,
            tc.tile_pool(name="ps1", bufs=5, space="PSUM") as ps1_pool,
            tc.tile_pool(name="ps2", bufs=3, space="PSUM") as ps2_pool,
        ):
            # interleave input DMAs so the earliest-needed tiles land first:
            # pair-0 D blocks and image-0 X blocks ahead of everything else
            d_t = [[None] * N_PAIR for _ in range(2)]
            x_t = [[None] * PER_CORE for _ in range(2)]

            def load_d(k, pr):
                if k == 0:
                    t = d_pool.tile([128, 512], _MM_DT, tag="d0", name=f"d_0_{pr}")
                    nc.sync.dma_start(out=t[:], in_=d_dram[:, pr * 512:(pr + 1) * 512])
                    d_t[0][pr] = t
                else:
                    t = d_pool.tile([128, 512], _MM_DT, tag="d1", name=f"d_1_{pr}")
                    d0 = d_t[0][pr]
                    for ha in range(2):
                        b = ha * 256
                        nc.vector.tensor_copy(
                            t[:, b:b + 128],
                            d0[:, b + 128:b + 256].bitcast(_F32),
                        )
                        nc.vector.tensor_copy(
                            t[:, b + 128:b + 256],
                            d0[:, b:b + 128].bitcast(_F32),
                        )
                    d_t[1][pr] = t

            def load_x(n):
                # one DMA per image: tile [p, k, h] <- x[n, k*128+p, h]
                t = xin_pool.tile([128, 2, SIZE], _MM_DT, tag="x", name=f"x_{n}")
                nc.gpsimd.dma_start(
                    out=t[:], in_=x_dram[n].rearrange("(k p) h -> p k h", k=2))
                x_t[0][n] = t

            load_d(0, 0)
            load_x(0)
            load_d(1, 0)
            for pr in range(1, N_PAIR):
                load_d(0, pr)
                load_d(1, pr)
            for n in range(1, PER_CORE):
                load_x(n)

            for n in range(PER_CORE):
                o1 = [None] * N_PAIR
                for pr in range(N_PAIR):
                    s1h = []
                    for m in range(2):
                        s1 = ps1_pool.tile([128, 512], _F32, tag="s1", name=f"s1_{n}_{pr}_{m}")
                        for k in range(2):
                            nc.tensor.matmul(
                                s1[:],
                                x_t[0][n][:, k, m * 128:(m + 1) * 128],
                                d_t[k][pr][:],
                                start=(k == 0),
                                stop=(k == 1),
                                skip_group_check=True,
                            )
                        s1h.append(s1)
                    o1t = o1_pool.tile([128, 1024], _MM_DT, tag="o1", name=f"o1_{n}_{pr}")
                    for m in range(2):
                        if (pr + m) % 2 == 0:
                            nc.vector.tensor_copy(o1t[:, m * 512:(m + 1) * 512], s1h[m][:])
                        else:
                            nc.scalar.copy(o1t[:, m * 512:(m + 1) * 512], s1h[m][:])
                    o1[pr] = o1t

                # two half-tiles per w-block: channels 0-3 from pairs 0-1,
                # channels 4-7 from pairs 2-3 -> DMA each half when ready
                last = n == PER_CORE - 1
                out_half = [
                    [
                        obig_pool.tile([128, N_U // 2, SIZE], _F32, tag="ob", name=f"ob_{n}_{m2b}_{hb}")
                        for hb in range(2)
                    ]
                    for m2b in range(2)
                ]
                for pr in range(N_PAIR):
                    hb = pr // 2
                    for m2 in range(2):
                        s2 = ps2_pool.tile([128, 2, SIZE], _F32, tag="s2", name=f"s2_{n}_{pr}_{m2}")
                        for ha in range(2):
                            for kp in range(2):
                                lhs = o1[pr][:, kp * 512 + ha * 256 + m2 * 128:
                                             kp * 512 + ha * 256 + (m2 + 1) * 128]
                                rhs = d_t[kp][pr][:, ha * 256:(ha + 1) * 256]
                                nc.tensor.matmul(
                                    s2[:, ha, :],
                                    lhs,
                                    rhs,
                                    start=(kp == 0),
                                    stop=(kp == 1),
                                    skip_group_check=True,
                                )
                        dst = out_half[m2][hb][:, (2 * pr) % 4:(2 * pr) % 4 + 2, :]
                        if m2 == 0:
                            nc.vector.tensor_copy(dst, s2[:])
                        else:
                            nc.scalar.copy(dst, s2[:])
                    if last:
                        for m2 in range(2):
                            nc.sync.dma_start(
                                out=o_dram[n, m2 * 128:(m2 + 1) * 128,
                                           2 * pr:2 * pr + 2, :],
                                in_=out_half[m2][hb][:, (2 * pr) % 4:(2 * pr) % 4 + 2, :],
                            )
                    elif pr % 2 == 1:
                        for m2 in range(2):
                            nc.sync.dma_start(
                                out=o_dram[n, m2 * 128:(m2 + 1) * 128,
                                           hb * 4:(hb + 1) * 4, :],
                                in_=out_half[m2][hb][:],
                            )

    _split_sync_waits(nc, max_waits=1)
    return nc


_CACHE = {}


def _ensure_neuron_backend():
    """If the caller pinned JAX_PLATFORMS=cpu (common for running the jax
    reference), re-open the accelerator platform for the bass run."""
    import jax

    try:
        if any(d.platform != "cpu" for d in jax.devices()):
            return
    except Exception:
        pass
    os.environ["JAX_PLATFORMS"] = ""
    try:
        from jax._src import xla_bridge

        xla_bridge._clear_backends()
        jax.devices()
    except Exception:
        pass



def _install_trace_shims():
    """Best-effort: let run_bass_kernel_spmd(trace=True) work in this
    container -- antenv.axon_hooks is missing from the image and the
    artifact bucket is unreachable; degrade both gracefully."""
    try:
        import antenv.axon_hooks  # noqa: F401
    except ImportError:
        import types

        hook = None
        try:
            if "/root/.axon_site" not in sys.path and os.path.isdir("/root/.axon_site"):
                sys.path.insert(0, "/root/.axon_site")
            from trn_agent_boot.trn_boot import _ntff_profile_via_ctypes

            hook = _ntff_profile_via_ctypes("/opt/axon/libaxon_pjrt.so")
        except Exception:
            hook = None
        mod = types.ModuleType("antenv.axon_hooks")
        mod.get_axon_ntff_profile_hook = lambda: hook
        mod.set_axon_ntff_profile_hook = lambda h: None
        sys.modules["antenv.axon_hooks"] = mod
    try:
        import concourse.bass_utils as bu

        if not getattr(bu.upload_artifacts, "_safe", False):
            _orig = bu.upload_artifacts

            def _safe_upload(tmpdir):
                try:
                    return _orig(tmpdir)
                except Exception:
                    return tmpdir

            _safe_upload._safe = True
            bu.upload_artifacts = _safe_upload
    except Exception:
        pass


def kernel(x: np.ndarray) -> np.ndarray:
    x = np.asarray(x)
    assert x.shape == (N_IMG, SIZE, SIZE, 1) and x.dtype == np.float32
    _ensure_neuron_backend()
    _install_trace_shims()
    if "nc" not in _CACHE:
        if ALGO == "g":
            _CACHE["nc"] = _build_program_g()
            _CACHE["g"], _CACHE["gt"] = _build_g_matrices()
        elif ALGO == "batch":
            _CACHE["nc"] = _build_program_batch()
            _CACHE["d"] = _build_d_matrices()
        elif ALGO == "fold":
            _CACHE["nc"] = _build_program_fold()
            _CACHE["d"] = _build_fold_d()
        else:
            _CACHE["nc"] = _build_program_direct()
            _CACHE["d"] = _build_d_matrices()
    nc = _CACHE["nc"]

    xs = x.reshape(N_IMG, SIZE, SIZE)
    if ALGO == "fold":
        in_maps = [
            {"x": _fold_x(xs[c * PER_CORE:(c + 1) * PER_CORE]), "d": _CACHE["d"]}
            for c in range(N_CORES)
        ]
        res = run_bass_kernel_spmd(nc, in_maps, list(range(N_CORES)))
        kernel._last_results = res
        out = np.empty((N_IMG, SIZE, SIZE, N_U + 1), np.float32)
        out[..., 0] = xs
        # channel 8 (u=127): D = I - n n^T with n the Nyquist vector; exact.
        nv = (((-1) ** np.arange(SIZE)) / 16.0).astype(np.float32)
        v = np.einsum("p,ipq->iq", nv, xs)          # n^T X   [img, 256]
        w = np.einsum("ipq,q->ip", xs, nv)          # X n     [img, 256]
        s = v @ nv                                   # n^T X n [img]
        out[..., 8] = (xs - nv[None, :, None] * v[:, None, :]
                       - w[:, :, None] * nv[None, None, :]
                       + s[:, None, None] * np.outer(nv, nv)[None])
        # device channels 1..7: unfold parity quadrants (outT on device)
        # res out per core: [n_ch, 2(b), 128(sb), img, 2(a), sa], fp8 + fp16
        def _core_arr(c):
            r16 = np.asarray(res.results[c]["out"]).astype(np.float32)
            r16 = r16.reshape(N_UD - N_F8, 2, 128, PER_CORE, 2, 128)
            if not N_F8:
                return r16
            r8 = np.asarray(res.results[c]["out8"]).astype(np.float32)
            return np.concatenate(
                [r8.reshape(N_F8, 2, 128, PER_CORE, 2, 128), r16], axis=0)
        arr = np.concatenate([_core_arr(c) for c in range(N_CORES)], axis=3)
        for ui in range(N_UD):
            acc = np.zeros((N_IMG, SIZE, SIZE), np.float32)
            for a, sa in ((0, 1.0), (1, -1.0)):
                for b, sb in ((0, 1.0), (1, -1.0)):
                    # [sb, img, sa] -> [img, sa, sb]
                    q = arr[ui, b, :, :, a, :].astype(np.float32).transpose(1, 2, 0)
                    acc[:, :128, :128] += q
                    acc[:, 128:, :128] += sa * q[:, ::-1, :]
                    acc[:, :128, 128:] += sb * q[:, :, ::-1]
                    acc[:, 128:, 128:] += (sa * sb) * q[:, ::-1, ::-1]
            out[..., ui + 1] = acc
        return out

    extras = {"d": _CACHE["d"]} if ALGO != "g" else {"g": _CACHE["g"], "gt": _CACHE["gt"]}
    in_maps = [
        {"x": np.ascontiguousarray(xs[c * PER_CORE:(c + 1) * PER_CORE]), **extras}
        for c in range(N_CORES)
    ]
    res = run_bass_kernel_spmd(nc, in_maps, list(range(N_CORES)))
    kernel._last_results = res
    out = np.empty((N_IMG, SIZE, SIZE, N_U + 1), np.float32)
    out[..., 0] = xs
    filt = np.concatenate([res.results[c]["out"] for c in range(N_CORES)], axis=0)
    if ALGO == "batch":
        # device output is [n, h, c, w] (transposed); reorder to [n, w, h, c]
        out[..., 1:] = filt.transpose(0, 3, 1, 2)
    else:
        # device output is [n, w, c, h] planar; reorder to [n, w, h, c]
        out[..., 1:] = filt.transpose(0, 1, 3, 2)
    return out

